# revision 49
# baseline (speedup 1.0000x reference)
"""GCNConv (N=100000, E=1.6M, 128->64) on 8 Trainium2 NeuronCores.

Aggregate-first formulation:  out = D^-1/2 (A+I) D^-1/2 (X W) + b
                                  = [D^-1/2 (A+I) (D^-1/2 X)] W + b
so the device gathers rows of the host-prescaled table xs = dis * x
(256B rows, bf16), scatter-adds them into per-destination-window
accumulators agg_T[fin, dest] via PE one-hot matmuls, THEN applies W:

  per dest window w (slot s on its core):
    agg_T[fin, d] =  sum_groups  msgs_g[tok, fin].T-contraction S_g[tok, d]
                   + selfx_s[t, fin].T-contraction diag(dis_s)[t, d]
    out[d, f]     = dis[d] * (agg_sb.T-contraction W)[d, f] + bias[f]

Token stream: edges (minus self-loops; the self term is the diag matmul)
bucketed per (slot, src chunk of 32768 rows) for int16 dma_gather
indices, padded to groups of 128 (pads gather row 0 of the chunk and
carry dest=255 so their one-hot column is zero).  Gathers are issued as
1024-idx single-packet calls round-robined over all 4 SWDGE queues --
each queue's descriptor generation runs on its own Q7 core pair, so 4
calls generate in parallel and the SDMA drain (~1ns/desc) is the floor.
Host does index-space preprocessing and the O(N*F) dis-scaling only;
all O(E*F) aggregation and the O(N*F*F') transform run on device.
"""
import numpy as np
import ml_dtypes

P = 128
FIN, FOUT = 128, 64
N = 100000
N_ROWS = 100352          # gather table rows (node v at row v; tail zero)
CHUNK = 32768            # int16 index reach per gather call
N_CHUNKS = 4
N_CORES = 8
SB_SLOTS = 4             # slots (dest windows) per superblock
NW = (N + P - 1) // P    # 782 dest windows
PAD_DEST = 255           # pad-token dest; never matches iota 0..127
GATHER_CALL = 1024       # idxs per dma_gather (queue-balance granularity)

BF16 = ml_dtypes.bfloat16


def preprocess(x, edge_index, weight, bias):
    row = np.asarray(edge_index[0]).astype(np.int32)
    col = np.asarray(edge_index[1]).astype(np.int32)
    deg = np.bincount(row, minlength=N).astype(np.float32)
    with np.errstate(divide="ignore"):
        dis = deg ** np.float32(-0.5)
    n_inf = int(np.isinf(dis).sum())

    keep = row != col
    er = row[keep]
    ec = col[keep]

    win = er // P
    bucket = (ec >> 15).astype(np.int32)

    cnt = np.zeros((NW, N_CHUNKS), dtype=np.int64)
    np.add.at(cnt, (win, bucket), 1)
    grp_wb = -(-cnt // P)
    g_w = grp_wb.sum(1)

    # LPT window -> core assignment, balancing total group counts.
    # Insertion order is big-first, which also aligns per-slot sizes
    # across cores (slot k holds each core's k-th biggest window).
    order = np.argsort(-g_w, kind="stable")
    core_tot = np.zeros(N_CORES, dtype=np.int64)
    core_of_win = np.zeros(NW, dtype=np.int32)
    core_wins = [[] for _ in range(N_CORES)]
    for w in order:
        c = int(np.argmin(core_tot))
        core_of_win[w] = c
        core_wins[c].append(w)
        core_tot[c] += g_w[w]
    S_SLOTS = max(len(ws) for ws in core_wins)
    slot_win = -np.ones((N_CORES, S_SLOTS), dtype=np.int64)
    for c in range(N_CORES):
        for s, w in enumerate(core_wins[c]):
            slot_win[c, s] = w

    # static per (slot, bucket) group counts = max over cores
    B = np.zeros((S_SLOTS, N_CHUNKS), dtype=np.int64)
    for c in range(N_CORES):
        for s in range(S_SLOTS):
            w = slot_win[c, s]
            if w >= 0:
                B[s] = np.maximum(B[s], grp_wb[w])

    # token layout: superblocks of SB_SLOTS slots, inside ordered
    # (bucket, slot); one gather call range per (superblock, bucket)
    n_sb = -(-S_SLOTS // SB_SLOTS)
    tok_off = np.zeros((S_SLOTS, N_CHUNKS), dtype=np.int64)
    sb_tok_off = np.zeros(n_sb + 1, dtype=np.int64)
    call_info = []
    t = 0
    for isb in range(n_sb):
        sb_tok_off[isb] = t
        slots = range(isb * SB_SLOTS, min((isb + 1) * SB_SLOTS, S_SLOTS))
        calls = []
        for b in range(N_CHUNKS):
            cb = t
            for s in slots:
                tok_off[s, b] = t
                t += B[s, b] * P
            if t > cb:
                calls.append((b, cb, t - cb))
        call_info.append(calls)
    sb_tok_off[n_sb] = t
    T_TOT = t
    G_TOT = T_TOT // P

    # pads: idx=-1 at each run's tail; calls are per-run so the gather
    # ucode trims them (num_idxs_reg = per-core real count).  Their gt
    # lanes stay stale-but-finite (bufs memset once); S column is zero.
    idx_all = np.full((N_CORES, T_TOT), -1, dtype=np.int16)
    dest_all = np.full((N_CORES, T_TOT), PAD_DEST, dtype=np.int16)

    slot_of_win = np.full(NW, -1, dtype=np.int64)
    for c in range(N_CORES):
        slot_of_win[:] = -1
        for s in range(S_SLOTS):
            w = slot_win[c, s]
            if w >= 0:
                slot_of_win[w] = s
        m = core_of_win[win] == c
        e_s = slot_of_win[win[m]]
        e_b = bucket[m]
        e_u = ec[m].astype(np.int64)
        e_dr = (er[m] % P).astype(np.int16)
        key = (e_s * N_CHUNKS + e_b) * np.int64(N_ROWS + 1) + e_u
        sort = np.argsort(key, kind="stable")
        e_s, e_b, e_u, e_dr = e_s[sort], e_b[sort], e_u[sort], e_dr[sort]
        sb_sorted = e_s * N_CHUNKS + e_b
        change = np.flatnonzero(np.diff(sb_sorted)) + 1
        starts = np.concatenate([[0], change])
        run_id = np.zeros(len(sb_sorted), dtype=np.int64)
        run_id[change] = 1
        run_id = np.cumsum(run_id)
        within = np.arange(len(sb_sorted)) - starts[run_id]
        pos = tok_off[e_s, e_b] + within
        idx_all[c, pos] = (e_u - e_b * CHUNK).astype(np.int16)
        dest_all[c, pos] = e_dr

    idx_dev = np.empty((N_CORES, 128, T_TOT // 16), dtype=np.int16)
    dest_dev = np.empty((N_CORES, 128, G_TOT), dtype=BF16)
    for c in range(N_CORES):
        idx_dev[c] = np.tile(idx_all[c].reshape(T_TOT // 16, 16).T, (8, 1))
        dest_dev[c] = dest_all[c].reshape(G_TOT, 128).T.astype(BF16)

    # gather call list: one per <=1024-idx span of each (slot, bucket)
    # run so each core's -1 pad tail is trimmed by the ucode; per-core
    # real counts ship in rlen (consumed 8-at-a-time into registers)
    real_cnt = np.zeros((N_CORES, S_SLOTS, N_CHUNKS), dtype=np.int64)
    for c in range(N_CORES):
        for s in range(S_SLOTS):
            w = slot_win[c, s]
            if w >= 0:
                real_cnt[c, s] = cnt[w]
    calls = []
    for isb in range(n_sb):
        slots = range(isb * SB_SLOTS, min((isb + 1) * SB_SLOTS, S_SLOTS))
        for b in range(N_CHUNKS):
            for s in slots:
                nrun = int(B[s, b]) * P
                for sub in range(0, nrun, GATHER_CALL):
                    n = min(GATHER_CALL, nrun - sub)
                    calls.append((isb, b, int(tok_off[s, b]) + sub, n,
                                  s, sub))
    NCALLS = len(calls)
    NC8 = -(-NCALLS // 8) * 8
    rlen_dev = np.zeros((N_CORES, 1, NC8), dtype=np.int32)
    for i, (isb, b, off, n, s, sub) in enumerate(calls):
        for c in range(N_CORES):
            rlen_dev[c, 0, i] = min(max(int(real_cnt[c, s, b]) - sub, 0), n)

    dis_dev = np.zeros((N_CORES, 128, S_SLOTS), dtype=np.float32)
    for c in range(N_CORES):
        for s in range(S_SLOTS):
            w = slot_win[c, s]
            if w >= 0:
                lo = w * P
                hi = min(lo + P, N)
                dis_dev[c, : hi - lo, s] = dis[lo:hi]

    xs = np.asarray(x, dtype=np.float32) * dis[:, None]
    if n_inf:
        xs = np.nan_to_num(xs, nan=0.0, posinf=0.0, neginf=0.0)
    xs_dev = np.zeros((N_ROWS, FIN), dtype=BF16)
    xs_dev[:N] = xs.astype(BF16)

    # per-core self rows, partition-major so one big DMA preloads them all:
    # selfx_dev[c][p, s*FIN+e] = xs[win(c,s)*128 + p, e].  The self matmul
    # rhs is a plain identity (xs carries one dis factor, the flush the
    # other); invalid dests have dis=0 so no masking is needed.
    selfx_dev = np.zeros((N_CORES, P, S_SLOTS * FIN), dtype=BF16)
    for c in range(N_CORES):
        for s in range(S_SLOTS):
            w = slot_win[c, s]
            if w >= 0:
                lo = w * P
                hi = min(lo + P, N)
                selfx_dev[c, : hi - lo, s * FIN: (s + 1) * FIN] = xs_dev[lo:hi]
    ident = np.eye(P, dtype=np.float32).astype(BF16)

    w_dev = np.asarray(weight, dtype=np.float32).astype(BF16)
    bias_dev = np.tile(np.asarray(bias, dtype=np.float32), (P, 1))
    iota = np.tile(np.arange(P, dtype=np.float32).astype(BF16), (P, 1))

    return dict(
        S_SLOTS=S_SLOTS, B=B, n_sb=n_sb, tok_off=tok_off,
        sb_tok_off=sb_tok_off, call_info=call_info, T_TOT=T_TOT, G_TOT=G_TOT,
        slot_win=slot_win, idx_dev=idx_dev, dest_dev=dest_dev,
        dis_dev=dis_dev, xs_dev=xs_dev, selfx_dev=selfx_dev,
        ident=ident, w_dev=w_dev, bias_dev=bias_dev, iota=iota,
        calls=calls, NCALLS=NCALLS, NC8=NC8, rlen_dev=rlen_dev,
    )


def build_bass(pp):
    import concourse.bacc as bacc
    import concourse.tile as tile
    from concourse import mybir

    dt = mybir.dt
    S_SLOTS, B = pp["S_SLOTS"], pp["B"]
    T_TOT, G_TOT, n_sb = pp["T_TOT"], pp["G_TOT"], pp["n_sb"]
    sb_tok_off, tok_off = pp["sb_tok_off"], pp["tok_off"]
    calls, NCALLS = pp["calls"], pp["NCALLS"]
    TSB_MAX = int(np.diff(sb_tok_off).max())
    NC8 = pp["NC8"]
    calls_by_sb = [[] for _ in range(n_sb)]
    for ci, (isb, b, off, n, s, sub) in enumerate(calls):
        calls_by_sb[isb].append((ci, b, off, n))

    nc = bacc.Bacc("TRN2", target_bir_lowering=False, debug=False,
                   num_devices=N_CORES, num_swdge_queues=4)
    xs_d = nc.dram_tensor("xs", [N_ROWS, FIN], dt.bfloat16, kind="ExternalInput")
    w_d = nc.dram_tensor("w", [FIN, FOUT], dt.bfloat16, kind="ExternalInput")
    bias_d = nc.dram_tensor("bias", [P, FOUT], dt.float32, kind="ExternalInput")
    idx_d = nc.dram_tensor("idx", [128, T_TOT // 16], dt.int16, kind="ExternalInput")
    dest_d = nc.dram_tensor("dest", [P, G_TOT], dt.bfloat16, kind="ExternalInput")
    dis_d = nc.dram_tensor("dis", [P, S_SLOTS], dt.float32, kind="ExternalInput")
    selfx_d = nc.dram_tensor("selfx", [P, S_SLOTS * FIN], dt.bfloat16,
                             kind="ExternalInput")
    ident_d = nc.dram_tensor("ident", [P, P], dt.bfloat16, kind="ExternalInput")
    iota_d = nc.dram_tensor("iota", [P, P], dt.bfloat16, kind="ExternalInput")
    rlen_d = nc.dram_tensor("rlen", [1, NC8], dt.int32, kind="ExternalInput")
    out_d = nc.dram_tensor("out", [P, S_SLOTS * FOUT], dt.float32,
                           kind="ExternalOutput")

    qn = [0]  # rotating SWDGE queue assignment

    with tile.TileContext(nc) as tc:
        with tc.tile_pool(name="const", bufs=1) as cpool:
            # idx first (gates the first gathers), split across both HWDGE
            # queues; everything else behind it
            idx_t = cpool.tile([128, T_TOT // 16], dt.int16)
            q4 = ((T_TOT // 16) // 4) & ~15
            nc.sync.dma_start(out=idx_t[:, :q4], in_=idx_d.ap()[:, :q4])
            nc.scalar.dma_start(out=idx_t[:, q4: 2 * q4],
                                in_=idx_d.ap()[:, q4: 2 * q4])
            nc.sync.dma_start(out=idx_t[:, 2 * q4: 3 * q4],
                              in_=idx_d.ap()[:, 2 * q4: 3 * q4])
            nc.scalar.dma_start(out=idx_t[:, 3 * q4:],
                                in_=idx_d.ap()[:, 3 * q4:])
            w_t = cpool.tile([FIN, FOUT], dt.bfloat16)
            nc.sync.dma_start(out=w_t[:], in_=w_d.ap())
            bias_t = cpool.tile([P, FOUT], dt.float32)
            nc.scalar.dma_start(out=bias_t[:], in_=bias_d.ap())
            dis_t = cpool.tile([P, S_SLOTS], dt.float32)
            nc.scalar.dma_start(out=dis_t[:], in_=dis_d.ap())
            iota_t = cpool.tile([P, P], dt.bfloat16)
            nc.scalar.dma_start(out=iota_t[:], in_=iota_d.ap())
            dest_t = cpool.tile([P, G_TOT], dt.bfloat16)
            nc.scalar.dma_start(out=dest_t[:], in_=dest_d.ap())
            ident_t = cpool.tile([P, P], dt.bfloat16)
            nc.scalar.dma_start(out=ident_t[:], in_=ident_d.ap())
            selfx_t = cpool.tile([P, S_SLOTS * FIN], dt.bfloat16)
            nc.sync.dma_start(out=selfx_t[:], in_=selfx_d.ap())
            rlen_t = cpool.tile([1, NC8], dt.int32)
            nc.scalar.dma_start(out=rlen_t[:], in_=rlen_d.ap())
            # per-core real call lengths, loaded 8 registers per
            # TENSOR_LOAD so the 4-deep Pool exec queue stays gather-heavy
            rregs = [nc.gpsimd.alloc_register(f"rl{i}") for i in range(8)]

            with tc.tile_pool(name="gt", bufs=3) as gpool, \
                 tc.tile_pool(name="st", bufs=2) as spool, \
                 tc.tile_pool(name="agg", bufs=4) as apool, \
                 tc.tile_pool(name="ob", bufs=3) as opool, \
                 tc.tile_pool(name="psA", bufs=4, space="PSUM") as psa, \
                 tc.tile_pool(name="psB", bufs=4, space="PSUM") as psb:
                # one-hot S build; chunked <=32 groups per DVE op (a
                # monolithic op over ~85 groups corrupts SBUF on HW).
                # Built ONE SUPERBLOCK AHEAD so the in-order DVE queue
                # doesn't serialize S(k+1) behind flush(k) -> matmuls(k).
                def build_S(j):
                    jt0, jt1 = int(sb_tok_off[j]), int(sb_tok_off[j + 1])
                    jG = (jt1 - jt0) // P
                    jg0 = jt0 // P
                    s_t = spool.tile([P, TSB_MAX], dt.bfloat16, tag="st")
                    for gch in range(0, jG, 32):
                        gn = min(32, jG - gch)
                        nc.vector.tensor_tensor(
                            out=s_t[:, gch * P: (gch + gn) * P]
                                .rearrange("p (g e) -> p g e", e=P),
                            in0=dest_t[:, jg0 + gch: jg0 + gch + gn]
                                .rearrange("p (g o) -> p g o", o=1)
                                .to_broadcast([P, gn, P]),
                            in1=iota_t[:].rearrange("p (o e) -> p o e", o=1)
                                .to_broadcast([P, gn, P]),
                            op=mybir.AluOpType.is_equal,
                        )
                    return s_t

                s_next = build_S(0)
                for isb in range(n_sb):
                    t0, t1 = int(sb_tok_off[isb]), int(sb_tok_off[isb + 1])
                    slots = range(isb * SB_SLOTS,
                                  min((isb + 1) * SB_SLOTS, S_SLOTS))
                    ns = len(slots)

                    gt = gpool.tile([P, TSB_MAX], dt.bfloat16, tag="gt")
                    gt3 = gt[:].rearrange("p (b e) -> p b e", e=FIN)
                    if isb < 3:
                        # trimmed pad lanes keep stale data: zero each
                        # buf once so stale is always finite bf16
                        nc.vector.memset(gt[:], 0)
                    for (ci, b, off, n) in calls_by_sb[isb]:
                        if ci % 8 == 0:
                            nc.gpsimd.reg_load(
                                rregs, rlen_t[0:1, ci: ci + 8])
                        hi_row = min((b + 1) * CHUNK, N_ROWS)
                        rel = off - t0
                        nc.gpsimd.dma_gather(
                            out_ap=gt3[:, rel // P: (rel + n) // P, :],
                            in_ap=xs_d.ap()[b * CHUNK: hi_row, :],
                            idxs_ap=idx_t[:, off // 16: (off + n) // 16],
                            num_idxs=n,
                            num_idxs_reg=rregs[ci % 8],
                            elem_size=FIN,
                            single_packet=True,
                            queue_num=qn[0],
                        )
                        qn[0] = (qn[0] + 1) % 4

                    s_t = s_next
                    if isb + 1 < n_sb:
                        s_next = build_S(isb + 1)

                    out_sb = opool.tile([P, SB_SLOTS * FOUT], dt.float32,
                                        tag="osb")
                    for si, s in enumerate(slots):
                        agg_ps = psa.tile([P, P], dt.float32, tag="agg")
                        k = 0
                        n_mm = int(B[s].sum())
                        for b in range(N_CHUNKS):
                            for g in range(int(B[s, b])):
                                blk = (int(tok_off[s, b]) - t0) // P + g
                                nc.tensor.matmul(
                                    out=agg_ps[:],
                                    lhsT=gt3[:, blk: blk + 1, :]
                                        .rearrange("p b e -> p (b e)"),
                                    rhs=s_t[:, blk * P: (blk + 1) * P],
                                    start=(k == 0), stop=False,
                                )
                                k += 1
                        nc.tensor.matmul(
                            out=agg_ps[:],
                            lhsT=selfx_t[:, s * FIN: (s + 1) * FIN],
                            rhs=ident_t[:],
                            start=(n_mm == 0), stop=True,
                        )
                        agg_sb = apool.tile([P, P], dt.bfloat16, tag="aggs")
                        nc.scalar.activation(
                            out=agg_sb[:], in_=agg_ps[:],
                            func=mybir.ActivationFunctionType.Copy)
                        out_ps = psb.tile([P, FOUT], dt.float32, tag="ops")
                        nc.tensor.matmul(
                            out=out_ps[:], lhsT=agg_sb[:], rhs=w_t[:],
                            start=True, stop=True,
                        )
                        nc.vector.scalar_tensor_tensor(
                            out=out_sb[:, si * FOUT: (si + 1) * FOUT],
                            in0=out_ps[:],
                            scalar=dis_t[:, s: s + 1],
                            in1=bias_t[:],
                            op0=mybir.AluOpType.mult,
                            op1=mybir.AluOpType.add,
                        )
                    nc.sync.dma_start(
                        out=out_d.ap()[:, slots.start * FOUT:
                                       (slots.start + ns) * FOUT],
                        in_=out_sb[:, : ns * FOUT])

    nc.compile()
    return nc


def assemble(pp, shards):
    out = np.zeros((N, FOUT), dtype=np.float32)
    for c in range(N_CORES):
        for s in range(pp["S_SLOTS"]):
            w = pp["slot_win"][c, s]
            if w < 0:
                continue
            lo = w * P
            hi = min(lo + P, N)
            out[lo:hi] = shards[c][: hi - lo, s * FOUT: (s + 1) * FOUT]
    return out


def make_in_maps(pp):
    in_maps = []
    for c in range(N_CORES):
        in_maps.append({
            "xs": pp["xs_dev"], "w": pp["w_dev"], "bias": pp["bias_dev"],
            "idx": pp["idx_dev"][c], "dest": pp["dest_dev"][c],
            "dis": pp["dis_dev"][c], "selfx": pp["selfx_dev"][c],
            "ident": pp["ident"], "iota": pp["iota"],
            "rlen": pp["rlen_dev"][c],
        })
    return in_maps


_CACHE = {}


def kernel(x, edge_index, weight, bias):
    from concourse import bass_utils

    pp = preprocess(x, edge_index, weight, bias)
    key = (pp["T_TOT"], pp["S_SLOTS"], pp["B"].tobytes())
    nc = _CACHE.get(key)
    if nc is None:
        nc = build_bass(pp)
        _CACHE[key] = nc

    res = bass_utils.run_bass_kernel_spmd(nc, make_in_maps(pp),
                                          core_ids=list(range(N_CORES)))
    shards = [res.results[c]["out"] for c in range(N_CORES)]
    return assemble(pp, shards)


# revision 58
# speedup vs baseline: 1.1646x; 1.1646x over previous
"""GCNConv (N=100000, E=1.6M, 128->64) on 8 Trainium2 NeuronCores.

Aggregate-first formulation:  out = D^-1/2 (A+I) D^-1/2 (X W) + b
                                  = [D^-1/2 (A+I) (D^-1/2 X)] W + b
so the device gathers rows of the host-prescaled table xs = dis * x
(256B rows, bf16), scatter-adds them into per-destination-window
accumulators agg_T[fin, dest] via PE one-hot matmuls, THEN applies W:

  per dest window w (slot s on its core):
    agg_T[fin, d] =  sum_groups  msgs_g[tok, fin].T-contraction S_g[tok, d]
                   + selfx_s[t, fin].T-contraction diag(dis_s)[t, d]
    out[d, f]     = dis[d] * (agg_sb.T-contraction W)[d, f] + bias[f]

Token stream: edges (minus self-loops; the self term is the diag matmul)
bucketed per (slot, src chunk of 32768 rows) for int16 dma_gather
indices, padded to groups of 128 (pads gather row 0 of the chunk and
carry dest=255 so their one-hot column is zero).  Gathers are issued as
1024-idx single-packet calls round-robined over all 4 SWDGE queues --
each queue's descriptor generation runs on its own Q7 core pair, so 4
calls generate in parallel and the SDMA drain (~1ns/desc) is the floor.
Host does index-space preprocessing and the O(N*F) dis-scaling only;
all O(E*F) aggregation and the O(N*F*F') transform run on device.
"""
import numpy as np
import ml_dtypes

P = 128
FIN, FOUT = 128, 64
N = 100000
N_ROWS = 100352          # gather table rows (node v at row v; tail zero)
CHUNK = 32768            # int16 index reach per gather call
N_CHUNKS = 4
N_CORES = 8
SB_SLOTS = 4             # slots (dest windows) per superblock
NW = (N + P - 1) // P    # 782 dest windows
PAD_DEST = 255           # pad-token dest; never matches iota 0..127
GATHER_CALL = 1024       # idxs per dma_gather (queue-balance granularity)

BF16 = ml_dtypes.bfloat16


def preprocess(x, edge_index, weight, bias):
    row = np.asarray(edge_index[0]).astype(np.int32)
    col = np.asarray(edge_index[1]).astype(np.int32)
    deg = np.bincount(row, minlength=N).astype(np.float32)
    with np.errstate(divide="ignore"):
        dis = deg ** np.float32(-0.5)
    n_inf = int(np.isinf(dis).sum())

    keep = row != col
    er = row[keep]
    ec = col[keep]

    win = er // P
    bucket = (ec >> 15).astype(np.int32)

    cnt = np.zeros((NW, N_CHUNKS), dtype=np.int64)
    np.add.at(cnt, (win, bucket), 1)
    grp_wb = -(-cnt // P)
    g_w = grp_wb.sum(1)

    # LPT window -> core assignment, balancing total group counts.
    # Insertion order is big-first, which also aligns per-slot sizes
    # across cores (slot k holds each core's k-th biggest window).
    order = np.argsort(-g_w, kind="stable")
    core_tot = np.zeros(N_CORES, dtype=np.int64)
    core_of_win = np.zeros(NW, dtype=np.int32)
    core_wins = [[] for _ in range(N_CORES)]
    for w in order:
        c = int(np.argmin(core_tot))
        core_of_win[w] = c
        core_wins[c].append(w)
        core_tot[c] += g_w[w]
    S_SLOTS = max(len(ws) for ws in core_wins)
    slot_win = -np.ones((N_CORES, S_SLOTS), dtype=np.int64)
    for c in range(N_CORES):
        for s, w in enumerate(core_wins[c]):
            slot_win[c, s] = w

    # static per (slot, bucket) group counts = max over cores
    B = np.zeros((S_SLOTS, N_CHUNKS), dtype=np.int64)
    for c in range(N_CORES):
        for s in range(S_SLOTS):
            w = slot_win[c, s]
            if w >= 0:
                B[s] = np.maximum(B[s], grp_wb[w])

    # token layout: superblocks of SB_SLOTS slots, inside ordered
    # (bucket, slot); one gather call range per (superblock, bucket)
    n_sb = -(-S_SLOTS // SB_SLOTS)
    tok_off = np.zeros((S_SLOTS, N_CHUNKS), dtype=np.int64)
    sb_tok_off = np.zeros(n_sb + 1, dtype=np.int64)
    call_info = []
    t = 0
    for isb in range(n_sb):
        sb_tok_off[isb] = t
        slots = range(isb * SB_SLOTS, min((isb + 1) * SB_SLOTS, S_SLOTS))
        calls = []
        for b in range(N_CHUNKS):
            cb = t
            for s in slots:
                tok_off[s, b] = t
                t += B[s, b] * P
            if t > cb:
                calls.append((b, cb, t - cb))
        call_info.append(calls)
    sb_tok_off[n_sb] = t
    T_TOT = t
    G_TOT = T_TOT // P

    # pads gather row 0 of their chunk (valid address; the one-hot S
    # column for dest=255 is all-zero so they contribute nothing)
    idx_all = np.zeros((N_CORES, T_TOT), dtype=np.int16)
    dest_all = np.full((N_CORES, T_TOT), PAD_DEST, dtype=np.int16)

    slot_of_win = np.full(NW, -1, dtype=np.int64)
    for c in range(N_CORES):
        slot_of_win[:] = -1
        for s in range(S_SLOTS):
            w = slot_win[c, s]
            if w >= 0:
                slot_of_win[w] = s
        m = core_of_win[win] == c
        e_s = slot_of_win[win[m]]
        e_b = bucket[m]
        e_u = ec[m].astype(np.int64)
        e_dr = (er[m] % P).astype(np.int16)
        key = (e_s * N_CHUNKS + e_b) * np.int64(N_ROWS + 1) + e_u
        sort = np.argsort(key, kind="stable")
        e_s, e_b, e_u, e_dr = e_s[sort], e_b[sort], e_u[sort], e_dr[sort]
        sb_sorted = e_s * N_CHUNKS + e_b
        change = np.flatnonzero(np.diff(sb_sorted)) + 1
        starts = np.concatenate([[0], change])
        run_id = np.zeros(len(sb_sorted), dtype=np.int64)
        run_id[change] = 1
        run_id = np.cumsum(run_id)
        within = np.arange(len(sb_sorted)) - starts[run_id]
        pos = tok_off[e_s, e_b] + within
        idx_all[c, pos] = (e_u - e_b * CHUNK).astype(np.int16)
        dest_all[c, pos] = e_dr

    idx_dev = np.empty((N_CORES, 128, T_TOT // 16), dtype=np.int16)
    dest_dev = np.empty((N_CORES, 128, G_TOT), dtype=BF16)
    for c in range(N_CORES):
        idx_dev[c] = np.tile(idx_all[c].reshape(T_TOT // 16, 16).T, (8, 1))
        dest_dev[c] = dest_all[c].reshape(G_TOT, 128).T.astype(BF16)

    # gather call list: one per <=1024-idx span of each (superblock,
    # bucket) range — minimal call count (per-call fixed cost dominates).
    # The LAST call of each range ends with the last slot's pad tail:
    # mark those pads idx=-1 and pass the per-core real length via a
    # register so the ucode trims them (free descs on gen AND drain).
    calls = []
    for isb in range(n_sb):
        slots = range(isb * SB_SLOTS, min((isb + 1) * SB_SLOTS, S_SLOTS))
        s_last = slots[-1]
        for (b, cb, ntok) in call_info[isb]:
            for sub in range(0, ntok, GATHER_CALL):
                n = min(GATHER_CALL, ntok - sub)
                last = sub + GATHER_CALL >= ntok
                calls.append((isb, b, cb + sub, n,
                              s_last if last else -1))
    NCALLS = len(calls)
    # per-core real lengths for ragged (range-tail) calls, 8-padded
    ragged = [i for i, c in enumerate(calls) if c[4] >= 0]
    NR8 = max(8, -(-len(ragged) // 8) * 8)
    rlen_dev = np.zeros((N_CORES, 1, NR8), dtype=np.int32)
    for ri, i in enumerate(ragged):
        isb, b, off, n, s_last = calls[i]
        for c in range(N_CORES):
            w = slot_win[c, s_last]
            real_end = int(tok_off[s_last, b]) + (
                int(cnt[w, b]) if w >= 0 else 0)
            rl = min(max(real_end - off, 0), n)
            rlen_dev[c, 0, ri] = rl
            # mark this core's trailing pads in this call as -1
            idx_all[c, off + rl: off + n] = -1
    NCALLS_R = len(ragged)
    # rebuild wrapped idx AFTER the -1 marking
    for c in range(N_CORES):
        idx_dev[c] = np.tile(idx_all[c].reshape(T_TOT // 16, 16).T, (8, 1))

    dis_dev = np.zeros((N_CORES, 128, S_SLOTS), dtype=np.float32)
    for c in range(N_CORES):
        for s in range(S_SLOTS):
            w = slot_win[c, s]
            if w >= 0:
                lo = w * P
                hi = min(lo + P, N)
                dis_dev[c, : hi - lo, s] = dis[lo:hi]

    xs = np.asarray(x, dtype=np.float32) * dis[:, None]
    if n_inf:
        xs = np.nan_to_num(xs, nan=0.0, posinf=0.0, neginf=0.0)
    xs_dev = np.zeros((N_ROWS, FIN), dtype=BF16)
    xs_dev[:N] = xs.astype(BF16)

    # per-core self rows, partition-major so one big DMA preloads them all:
    # selfx_dev[c][p, s*FIN+e] = xs[win(c,s)*128 + p, e].  The self matmul
    # rhs is a plain identity (xs carries one dis factor, the flush the
    # other); invalid dests have dis=0 so no masking is needed.
    selfx_dev = np.zeros((N_CORES, P, S_SLOTS * FIN), dtype=BF16)
    for c in range(N_CORES):
        for s in range(S_SLOTS):
            w = slot_win[c, s]
            if w >= 0:
                lo = w * P
                hi = min(lo + P, N)
                selfx_dev[c, : hi - lo, s * FIN: (s + 1) * FIN] = xs_dev[lo:hi]
    ident = np.eye(P, dtype=np.float32).astype(BF16)

    w_dev = np.asarray(weight, dtype=np.float32).astype(BF16)
    bias_dev = np.tile(np.asarray(bias, dtype=np.float32), (P, 1))
    iota = np.tile(np.arange(P, dtype=np.float32).astype(BF16), (P, 1))

    return dict(
        S_SLOTS=S_SLOTS, B=B, n_sb=n_sb, tok_off=tok_off,
        sb_tok_off=sb_tok_off, call_info=call_info, T_TOT=T_TOT, G_TOT=G_TOT,
        slot_win=slot_win, idx_dev=idx_dev, dest_dev=dest_dev,
        dis_dev=dis_dev, xs_dev=xs_dev, selfx_dev=selfx_dev,
        ident=ident, w_dev=w_dev, bias_dev=bias_dev, iota=iota,
        calls=calls, NCALLS=NCALLS, NR8=NR8, rlen_dev=rlen_dev,
    )


def build_bass(pp):
    import concourse.bacc as bacc
    import concourse.tile as tile
    from concourse import mybir

    dt = mybir.dt
    S_SLOTS, B = pp["S_SLOTS"], pp["B"]
    T_TOT, G_TOT, n_sb = pp["T_TOT"], pp["G_TOT"], pp["n_sb"]
    sb_tok_off, tok_off = pp["sb_tok_off"], pp["tok_off"]
    calls, NCALLS = pp["calls"], pp["NCALLS"]
    TSB_MAX = int(np.diff(sb_tok_off).max())
    NR8 = pp["NR8"]
    calls_by_sb = [[] for _ in range(n_sb)]
    ragged_ord = 0
    for (isb, b, off, n, s_last) in calls:
        ri = ragged_ord if s_last >= 0 else -1
        if s_last >= 0:
            ragged_ord += 1
        calls_by_sb[isb].append((b, off, n, ri))

    nc = bacc.Bacc("TRN2", target_bir_lowering=False, debug=False,
                   num_devices=N_CORES, num_swdge_queues=4)
    xs_d = nc.dram_tensor("xs", [N_ROWS, FIN], dt.bfloat16, kind="ExternalInput")
    w_d = nc.dram_tensor("w", [FIN, FOUT], dt.bfloat16, kind="ExternalInput")
    bias_d = nc.dram_tensor("bias", [P, FOUT], dt.float32, kind="ExternalInput")
    idx_d = nc.dram_tensor("idx", [128, T_TOT // 16], dt.int16, kind="ExternalInput")
    dest_d = nc.dram_tensor("dest", [P, G_TOT], dt.bfloat16, kind="ExternalInput")
    dis_d = nc.dram_tensor("dis", [P, S_SLOTS], dt.float32, kind="ExternalInput")
    selfx_d = nc.dram_tensor("selfx", [P, S_SLOTS * FIN], dt.bfloat16,
                             kind="ExternalInput")
    ident_d = nc.dram_tensor("ident", [P, P], dt.bfloat16, kind="ExternalInput")
    iota_d = nc.dram_tensor("iota", [P, P], dt.bfloat16, kind="ExternalInput")
    rlen_d = nc.dram_tensor("rlen", [1, NR8], dt.int32, kind="ExternalInput")
    out_d = nc.dram_tensor("out", [P, S_SLOTS * FOUT], dt.float32,
                           kind="ExternalOutput")

    qn = [0]  # rotating SWDGE queue assignment

    with tile.TileContext(nc) as tc:
        with tc.tile_pool(name="const", bufs=1) as cpool:
            # warm-up gather: absorbs the ~13us Q7 IRAM library load
            # while the idx table streams in
            wu_idx = cpool.tile([128, 4], dt.int16)
            nc.vector.memset(wu_idx[:], 0)
            wu_out = cpool.tile([P, 128], dt.bfloat16)
            nc.gpsimd.dma_gather(
                out_ap=wu_out[:].rearrange("p (b e) -> p b e", e=FIN),
                in_ap=xs_d.ap()[0:CHUNK, :],
                idxs_ap=wu_idx[:],
                num_idxs=64, num_idxs_reg=64, elem_size=FIN,
                single_packet=True, queue_num=0)
            # idx first (gates the first gathers), split across both HWDGE
            # queues; everything else behind it
            idx_t = cpool.tile([128, T_TOT // 16], dt.int16)
            q4 = ((T_TOT // 16) // 4) & ~15
            nc.sync.dma_start(out=idx_t[:, :q4], in_=idx_d.ap()[:, :q4])
            nc.scalar.dma_start(out=idx_t[:, q4: 2 * q4],
                                in_=idx_d.ap()[:, q4: 2 * q4])
            nc.sync.dma_start(out=idx_t[:, 2 * q4: 3 * q4],
                              in_=idx_d.ap()[:, 2 * q4: 3 * q4])
            nc.scalar.dma_start(out=idx_t[:, 3 * q4:],
                                in_=idx_d.ap()[:, 3 * q4:])
            w_t = cpool.tile([FIN, FOUT], dt.bfloat16)
            nc.sync.dma_start(out=w_t[:], in_=w_d.ap())
            bias_t = cpool.tile([P, FOUT], dt.float32)
            nc.scalar.dma_start(out=bias_t[:], in_=bias_d.ap())
            dis_t = cpool.tile([P, S_SLOTS], dt.float32)
            nc.scalar.dma_start(out=dis_t[:], in_=dis_d.ap())
            iota_t = cpool.tile([P, P], dt.bfloat16)
            nc.scalar.dma_start(out=iota_t[:], in_=iota_d.ap())
            dest_t = cpool.tile([P, G_TOT], dt.bfloat16)
            nc.scalar.dma_start(out=dest_t[:], in_=dest_d.ap())
            ident_t = cpool.tile([P, P], dt.bfloat16)
            nc.scalar.dma_start(out=ident_t[:], in_=ident_d.ap())
            selfx_t = cpool.tile([P, S_SLOTS * FIN], dt.bfloat16)
            nc.sync.dma_start(out=selfx_t[:], in_=selfx_d.ap())
            rlen_t = cpool.tile([1, NR8], dt.int32)
            nc.scalar.dma_start(out=rlen_t[:], in_=rlen_d.ap())
            rregs = [nc.gpsimd.alloc_register(f"rl{i}") for i in range(8)]

            with tc.tile_pool(name="gt", bufs=3) as gpool, \
                 tc.tile_pool(name="st", bufs=2) as spool, \
                 tc.tile_pool(name="agg", bufs=4) as apool, \
                 tc.tile_pool(name="ob", bufs=3) as opool, \
                 tc.tile_pool(name="psA", bufs=4, space="PSUM") as psa, \
                 tc.tile_pool(name="psB", bufs=4, space="PSUM") as psb:
                # one-hot S build; chunked <=32 groups per DVE op (a
                # monolithic op over ~85 groups corrupts SBUF on HW).
                # Built ONE SUPERBLOCK AHEAD so the in-order DVE queue
                # doesn't serialize S(k+1) behind flush(k) -> matmuls(k).
                def build_S(j):
                    jt0, jt1 = int(sb_tok_off[j]), int(sb_tok_off[j + 1])
                    jG = (jt1 - jt0) // P
                    jg0 = jt0 // P
                    s_t = spool.tile([P, TSB_MAX], dt.bfloat16, tag="st")
                    for gch in range(0, jG, 32):
                        gn = min(32, jG - gch)
                        nc.vector.tensor_tensor(
                            out=s_t[:, gch * P: (gch + gn) * P]
                                .rearrange("p (g e) -> p g e", e=P),
                            in0=dest_t[:, jg0 + gch: jg0 + gch + gn]
                                .rearrange("p (g o) -> p g o", o=1)
                                .to_broadcast([P, gn, P]),
                            in1=iota_t[:].rearrange("p (o e) -> p o e", o=1)
                                .to_broadcast([P, gn, P]),
                            op=mybir.AluOpType.is_equal,
                        )
                    return s_t

                s_next = build_S(0)
                for isb in range(n_sb):
                    t0, t1 = int(sb_tok_off[isb]), int(sb_tok_off[isb + 1])
                    slots = range(isb * SB_SLOTS,
                                  min((isb + 1) * SB_SLOTS, S_SLOTS))
                    ns = len(slots)

                    gt = gpool.tile([P, TSB_MAX], dt.bfloat16, tag="gt")
                    gt3 = gt[:].rearrange("p (b e) -> p b e", e=FIN)
                    if isb < 3:
                        # trimmed pad lanes keep stale data: zero each
                        # buf once so stale is always finite bf16
                        nc.vector.memset(gt[:], 0)
                    for (b, off, n, ri) in calls_by_sb[isb]:
                        hi_row = min((b + 1) * CHUNK, N_ROWS)
                        rel = off - t0
                        if ri >= 0 and ri % 8 == 0:
                            nc.gpsimd.reg_load(
                                rregs, rlen_t[0:1, ri: ri + 8])
                        nc.gpsimd.dma_gather(
                            out_ap=gt3[:, rel // P: (rel + n) // P, :],
                            in_ap=xs_d.ap()[b * CHUNK: hi_row, :],
                            idxs_ap=idx_t[:, off // 16: (off + n) // 16],
                            num_idxs=n,
                            num_idxs_reg=rregs[ri % 8] if ri >= 0 else n,
                            elem_size=FIN,
                            single_packet=True,
                            queue_num=qn[0],
                        )
                        qn[0] = (qn[0] + 1) % 4

                    s_t = s_next
                    if isb + 1 < n_sb:
                        s_next = build_S(isb + 1)

                    out_sb = opool.tile([P, SB_SLOTS * FOUT], dt.float32,
                                        tag="osb")
                    for si, s in enumerate(slots):
                        agg_ps = psa.tile([P, P], dt.float32, tag="agg")
                        k = 0
                        n_mm = int(B[s].sum())
                        for b in range(N_CHUNKS):
                            for g in range(int(B[s, b])):
                                blk = (int(tok_off[s, b]) - t0) // P + g
                                nc.tensor.matmul(
                                    out=agg_ps[:],
                                    lhsT=gt3[:, blk: blk + 1, :]
                                        .rearrange("p b e -> p (b e)"),
                                    rhs=s_t[:, blk * P: (blk + 1) * P],
                                    start=(k == 0), stop=False,
                                )
                                k += 1
                        nc.tensor.matmul(
                            out=agg_ps[:],
                            lhsT=selfx_t[:, s * FIN: (s + 1) * FIN],
                            rhs=ident_t[:],
                            start=(n_mm == 0), stop=True,
                        )
                        agg_sb = apool.tile([P, P], dt.bfloat16, tag="aggs")
                        nc.scalar.activation(
                            out=agg_sb[:], in_=agg_ps[:],
                            func=mybir.ActivationFunctionType.Copy)
                        out_ps = psb.tile([P, FOUT], dt.float32, tag="ops")
                        nc.tensor.matmul(
                            out=out_ps[:], lhsT=agg_sb[:], rhs=w_t[:],
                            start=True, stop=True,
                        )
                        nc.vector.scalar_tensor_tensor(
                            out=out_sb[:, si * FOUT: (si + 1) * FOUT],
                            in0=out_ps[:],
                            scalar=dis_t[:, s: s + 1],
                            in1=bias_t[:],
                            op0=mybir.AluOpType.mult,
                            op1=mybir.AluOpType.add,
                        )
                    nc.sync.dma_start(
                        out=out_d.ap()[:, slots.start * FOUT:
                                       (slots.start + ns) * FOUT],
                        in_=out_sb[:, : ns * FOUT])

    nc.compile()
    return nc


def assemble(pp, shards):
    out = np.zeros((N, FOUT), dtype=np.float32)
    for c in range(N_CORES):
        for s in range(pp["S_SLOTS"]):
            w = pp["slot_win"][c, s]
            if w < 0:
                continue
            lo = w * P
            hi = min(lo + P, N)
            out[lo:hi] = shards[c][: hi - lo, s * FOUT: (s + 1) * FOUT]
    return out


def make_in_maps(pp):
    in_maps = []
    for c in range(N_CORES):
        in_maps.append({
            "xs": pp["xs_dev"], "w": pp["w_dev"], "bias": pp["bias_dev"],
            "idx": pp["idx_dev"][c], "dest": pp["dest_dev"][c],
            "dis": pp["dis_dev"][c], "selfx": pp["selfx_dev"][c],
            "ident": pp["ident"], "iota": pp["iota"],
            "rlen": pp["rlen_dev"][c],
        })
    return in_maps


_CACHE = {}


def kernel(x, edge_index, weight, bias):
    from concourse import bass_utils

    pp = preprocess(x, edge_index, weight, bias)
    key = (pp["T_TOT"], pp["S_SLOTS"], pp["B"].tobytes())
    nc = _CACHE.get(key)
    if nc is None:
        nc = build_bass(pp)
        _CACHE[key] = nc

    res = bass_utils.run_bass_kernel_spmd(nc, make_in_maps(pp),
                                          core_ids=list(range(N_CORES)))
    shards = [res.results[c]["out"] for c in range(N_CORES)]
    return assemble(pp, shards)
